# revision 1
# baseline (speedup 1.0000x reference)
"""Trainium2 Bass kernel for nn_EvalEig: all eigenvalues of a batch of
16 = (4 batch x 4 angular-momentum) symmetric tridiagonal 2000x2000 matrices.

Matrix m (= 4*b + l):  diag[i] = 2*s + ptl[b,i] + l(l+1)/r_i^2,  offdiag = -s,
with s = (2000/100)^2 = 400, r_i = (i+1)*0.05.  Scaling the matrix by 1/s makes
the offdiagonal exactly -1; eigenvalues scale back by s.

Algorithm: Sturm-count bisection.  count(x) = # negative pivots of the LDL
factorization of (T' - x I), via the recurrence q_i = d_i - x - 1/q_{i-1}.
To express each step as one exact vector-engine reciprocal plus one
scalar_tensor_tensor ((in0 op0 scalar) op1 in1), the stored state alternates
sign:  s_i = -q_i on even i, s_i = q_i on odd i.  With r = 1/s_{i-1}:
    odd  i:  q_i = (r + d_i) - x     (eigenvalue-below iff q_i < 0)
    even i:  m_i = (r - d_i) + x     (eigenvalue-below iff m_i > 0)
Both indicator parities are reduced with one trick: pivots are written into a
wide rotating buffer; every G=8 steps the ACT engine takes Sign() of the whole
buffer in one op, and the PE array accumulates the signs into PSUM with +I
weights for even steps and -I weights for odd steps (stride-0 PSUM access
patterns make all chunks of one matmul land on the same accumulator columns).
Then count(x) = (steps + S)/2.  The exact (iterative-divide) reciprocal is
IEEE-clean at +-0/inf, so zero pivots recover exactly like the true recurrence.

Sharding: 8 cores x 2 matrices each. Per core, 2 matrices x 2000 eigenvalue
targets = 4000 bisection tasks laid out as [128 partitions, 32 free]:
partition p handles matrix (p // 64), eigenvalue indices (p%64)*32 + f.
Per-sweep cost ~1.5 ms/core exact / ~1.0 ms approx (2000 serial steps); the
final config (12 exact + 4 approx-tail sweeps) runs ~23.6 ms at rel error
2.2e-5 vs the reference (device-validated on the key(0) inputs).
"""
import numpy as np

RN = 2000
RM = 100.0
LMAX = 3
BDIM = 4
S = np.float32((RN / RM) ** 2)   # 400.0
NCORES = 8
MATS_PER_CORE = 2                # 16 matrices / 8 cores
PARTS_PER_MAT = 64               # 64 partitions per matrix
TASKS_PER_PART = 32              # 64*32 = 2048 >= 2000 eigenvalue slots
SWEEPS = 16
VARIANT = "v3"
APPROX = False
CHUNK = 8
APPROX_TAIL = 4

_CACHE = {}


def _build_nc(sweeps=SWEEPS, steps=RN, variant="v2", approx=False, chunk=16, wbufs=2, approx_tail=0):
    import concourse.bass as bass
    import concourse.mybir as mybir
    from concourse import bacc
    from concourse.tile import TileContext
    from concourse.masks import make_identity

    f32 = mybir.dt.float32
    bf16 = mybir.dt.bfloat16
    Alu = mybir.AluOpType

    nc = bacc.Bacc("TRN2", target_bir_lowering=False, debug=False)
    D = nc.dram_tensor("d", [128, steps], f32, kind="ExternalInput")
    K = nc.dram_tensor("ktgt", [128, TASKS_PER_PART], f32, kind="ExternalInput")
    LO0 = nc.dram_tensor("lo0", [128, TASKS_PER_PART], f32, kind="ExternalInput")
    HI0 = nc.dram_tensor("hi0", [128, TASKS_PER_PART], f32, kind="ExternalInput")
    EV = nc.dram_tensor("ev", [128, TASKS_PER_PART], f32, kind="ExternalOutput")

    W = TASKS_PER_PART
    with TileContext(nc) as tc:
        with (
            tc.tile_pool(name="const", bufs=1) as cpool,
            tc.tile_pool(name="state", bufs=1) as spool,
            tc.tile_pool(name="work", bufs=wbufs) as wpool,
            tc.tile_pool(name="psum", bufs=1, space="PSUM") as ppool,
        ):
            d_t = cpool.tile([128, steps], f32)
            nc.gpsimd.dma_start(d_t[:], D[:])
            k_t = cpool.tile([128, W], f32)
            nc.gpsimd.dma_start(k_t[:], K[:])
            id_t = cpool.tile([128, 128], bf16)
            make_identity(nc, id_t[:])
            idn_t = cpool.tile([128, 128], bf16)
            nc.vector.tensor_scalar_mul(idn_t[:], id_t[:], -1.0)

            lo_t = spool.tile([128, W], f32)
            nc.gpsimd.dma_start(lo_t[:], LO0[:])
            hi_t = spool.tile([128, W], f32)
            nc.gpsimd.dma_start(hi_t[:], HI0[:])

            def sweep_body_v3(_iv=None, use_approx=None):
                # mid = (lo+hi)/2
                t0 = wpool.tile([128, W], f32, tag="t0")
                nc.vector.tensor_add(t0[:], lo_t[:], hi_t[:])
                mid = wpool.tile([128, W], f32, tag="mid")
                nc.vector.tensor_scalar_mul(mid[:], t0[:], 0.5)

                spsum = ppool.tile([128, W], f32, tag="spsum")

                def psum_bc(ap, c):
                    ap2 = ap.copy()
                    ap2.ap = mybir.VecI64Pair([ap.ap[0], [0, c], ap.ap[1]])
                    return ap2

                def strided(ap, nblk, blk_stride_elems):
                    # [128, ...] slice -> [128, nblk, W] with given block stride
                    ap2 = ap.copy()
                    ap2.ap = mybir.VecI64Pair(
                        [ap.ap[0], [blk_stride_elems, nblk], [1, W]]
                    )
                    return ap2

                _apx = approx if use_approx is None else use_approx

                def recip(r_ap, m_ap):
                    if _apx:
                        nc.vector.reciprocal_approx_fast(out=r_ap, in_=m_ap)
                    else:
                        nc.vector.reciprocal(r_ap, m_ap)

                G = chunk
                Sign = mybir.ActivationFunctionType.Sign
                assert steps % G == 0 and G % 2 == 0
                # step 0 into slot 0 of the first wide pivot buffer
                mbuf = wpool.tile([128, W * G], f32, tag="mbuf")
                nc.vector.tensor_scalar(
                    mbuf[:, 0:W], mid[:], d_t[:, 0:1], None, op0=Alu.subtract
                )
                m_ap = mbuf[:, 0:W]
                for i in range(1, steps + 1):
                    j = i % G
                    if j == 0:
                        # buffer filled by step i-1: sign it, accumulate on PE
                        cb = wpool.tile([128, W * G], bf16, tag="cbuf")
                        nc.scalar.activation(cb[:], mbuf[:], Sign, scale=1.0)
                        ic = i // G
                        nc.tensor.matmul(
                            psum_bc(spsum[:], G // 2),
                            id_t[:],
                            strided(cb[:, :], G // 2, 2 * W),
                            start=(ic == 1),
                            stop=False,
                        )
                        nc.tensor.matmul(
                            psum_bc(spsum[:], G // 2),
                            idn_t[:],
                            strided(cb[:, W:], G // 2, 2 * W),
                            start=False,
                            stop=(i == steps),
                        )
                        if i == steps:
                            break
                        mbuf = wpool.tile([128, W * G], f32, tag="mbuf")
                    r = wpool.tile([128, W], f32, tag="r")
                    recip(r[:], m_ap)
                    m_ap = mbuf[:, j * W : (j + 1) * W]
                    if i % 2 == 1:
                        nc.vector.scalar_tensor_tensor(
                            m_ap, r[:], d_t[:, i : i + 1], mid[:],
                            op0=Alu.add, op1=Alu.subtract,
                        )
                    else:
                        nc.vector.scalar_tensor_tensor(
                            m_ap, r[:], d_t[:, i : i + 1], mid[:],
                            op0=Alu.subtract, op1=Alu.add,
                        )

                # count = (steps + S)/2 ; sel = count <= k -> lo=mid else hi=mid
                cntf = wpool.tile([128, W], f32, tag="cntf")
                nc.vector.tensor_scalar(
                    cntf[:], spsum[:], 0.5, steps * 0.5, op0=Alu.mult, op1=Alu.add
                )
                sel = wpool.tile([128, W], mybir.dt.uint8, tag="sel")
                nc.vector.tensor_tensor(sel[:], cntf[:], k_t[:], op=Alu.is_le)
                lo2 = wpool.tile([128, W], f32, tag="lo2")
                nc.vector.select(lo2[:], sel[:], mid[:], lo_t[:])
                hi2 = wpool.tile([128, W], f32, tag="hi2")
                nc.vector.select(hi2[:], sel[:], hi_t[:], mid[:])
                nc.vector.tensor_copy(lo_t[:], lo2[:])
                nc.vector.tensor_copy(hi_t[:], hi2[:])

            def sweep_body_v2(_iv=None):
                # mid = (lo+hi)/2
                t0 = wpool.tile([128, W], f32, tag="t0")
                nc.vector.tensor_add(t0[:], lo_t[:], hi_t[:])
                mid = wpool.tile([128, W], f32, tag="mid")
                nc.vector.tensor_scalar_mul(mid[:], t0[:], 0.5)

                spsum = ppool.tile([128, W], f32, tag="spsum")

                def psum_bcast(ap, c):
                    # [128, W] -> [128, c, W] with stride-0 middle dim, so one
                    # matmul over [128, c, W] accumulates all c chunks into the
                    # same PSUM columns.
                    ap2 = ap.copy()
                    ap2.ap = mybir.VecI64Pair([ap.ap[0], [0, c], ap.ap[1]])
                    return ap2

                _apx = approx if use_approx is None else use_approx

                def recip(r_ap, m_ap):
                    if _apx:
                        nc.vector.reciprocal_approx_fast(out=r_ap, in_=m_ap)
                    else:
                        nc.vector.reciprocal(r_ap, m_ap)

                Sign = mybir.ActivationFunctionType.Sign
                m = wpool.tile([128, W], f32, tag="m")
                nc.vector.tensor_scalar(
                    m[:], mid[:], d_t[:, 0:1], None, op0=Alu.subtract
                )
                cbuf = wpool.tile([128, W * chunk], bf16, tag="cbuf")
                # sign convention: s-step contribution is (1 + Sign(sigma*m))/2
                # with sigma=+1 on even (m-form) steps, -1 on odd (q-form) steps.
                nc.scalar.activation(cbuf[:, 0:W], m[:], Sign, scale=1.0)
                for i in range(1, steps):
                    r = wpool.tile([128, W], f32, tag="r")
                    recip(r[:], m[:])
                    m = wpool.tile([128, W], f32, tag="m")
                    if i % 2 == 1:
                        nc.vector.scalar_tensor_tensor(
                            m[:], r[:], d_t[:, i : i + 1], mid[:],
                            op0=Alu.add, op1=Alu.subtract,
                        )
                        sigma = -1.0
                    else:
                        nc.vector.scalar_tensor_tensor(
                            m[:], r[:], d_t[:, i : i + 1], mid[:],
                            op0=Alu.subtract, op1=Alu.add,
                        )
                        sigma = 1.0
                    j = i % chunk
                    nc.scalar.activation(
                        cbuf[:, j * W : (j + 1) * W], m[:], Sign, scale=sigma
                    )
                    if j == chunk - 1 or i == steps - 1:
                        ic = i // chunk
                        nchunk = j + 1
                        nc.tensor.matmul(
                            psum_bcast(spsum[:], nchunk),
                            id_t[:],
                            cbuf[:, 0 : nchunk * W].rearrange(
                                "p (c w) -> p c w", w=W
                            ),
                            start=(ic == 0),
                            stop=(i == steps - 1),
                        )
                        if i != steps - 1:
                            cbuf = wpool.tile([128, W * chunk], bf16, tag="cbuf")

                # count = (steps + S)/2 ; sel = count <= k -> lo = mid else hi = mid
                cntf = wpool.tile([128, W], f32, tag="cntf")
                nc.vector.tensor_scalar(
                    cntf[:], spsum[:], 0.5, steps * 0.5, op0=Alu.mult, op1=Alu.add
                )
                sel = wpool.tile([128, W], mybir.dt.uint8, tag="sel")
                nc.vector.tensor_tensor(sel[:], cntf[:], k_t[:], op=Alu.is_le)
                lo2 = wpool.tile([128, W], f32, tag="lo2")
                nc.vector.select(lo2[:], sel[:], mid[:], lo_t[:])
                hi2 = wpool.tile([128, W], f32, tag="hi2")
                nc.vector.select(hi2[:], sel[:], hi_t[:], mid[:])
                nc.vector.tensor_copy(lo_t[:], lo2[:])
                nc.vector.tensor_copy(hi_t[:], hi2[:])

            def sweep_body(_iv=None):
                # mid = (lo+hi)/2
                t0 = wpool.tile([128, W], f32, tag="t0")
                nc.vector.tensor_add(t0[:], lo_t[:], hi_t[:])
                mid = wpool.tile([128, W], f32, tag="mid")
                nc.vector.tensor_scalar_mul(mid[:], t0[:], 0.5)

                cnt = ppool.tile([128, W], f32, tag="cnt")

                def count_step(m_tile, cmp_op, first, last):
                    if variant == "nocount":
                        if first:
                            nc.vector.memset(cnt[:], 0.0)
                        return
                    if variant == "vecmp":
                        c = wpool.tile([128, W], bf16, tag="c")
                        nc.vector.tensor_scalar(c[:], m_tile[:], 0.0, None, op0=cmp_op)
                    else:
                        c = wpool.tile([128, W], bf16, tag="c")
                        nc.gpsimd.tensor_scalar(c[:], m_tile[:], 0.0, None, op0=cmp_op)
                    if variant == "nomatmul":
                        if first:
                            nc.vector.memset(cnt[:], 0.0)
                        return
                    nc.tensor.matmul(cnt[:], id_t[:], c[:], start=first, stop=last)

                # step 0: m = mid - d_0
                m = wpool.tile([128, W], f32, tag="m")
                nc.vector.tensor_scalar(
                    m[:], mid[:], d_t[:, 0:1], None, op0=Alu.subtract
                )
                count_step(m, Alu.is_gt, True, False)

                # Alternating forms: state s_i = m_i = -q_i on even i,
                # s_i = q_i on odd i.  With r = 1/s_{i-1}:
                #   odd  i: q_i = (r + d_i) - x   (count q_i < 0)
                #   even i: m_i = (r - d_i) + x   (count m_i > 0)
                for i in range(1, steps):
                    r = wpool.tile([128, W], f32, tag="r")
                    nc.vector.reciprocal(r[:], m[:])
                    m = wpool.tile([128, W], f32, tag="m")
                    if i % 2 == 1:
                        nc.vector.scalar_tensor_tensor(
                            m[:], r[:], d_t[:, i : i + 1], mid[:],
                            op0=Alu.add, op1=Alu.subtract,
                        )
                        cmp_op = Alu.is_lt
                    else:
                        nc.vector.scalar_tensor_tensor(
                            m[:], r[:], d_t[:, i : i + 1], mid[:],
                            op0=Alu.subtract, op1=Alu.add,
                        )
                        cmp_op = Alu.is_gt
                    count_step(m, cmp_op, False, i == steps - 1)

                # bisection update: sel = (cnt <= k) -> lo = mid else hi = mid
                sel = wpool.tile([128, W], mybir.dt.uint8, tag="sel")
                nc.vector.tensor_tensor(sel[:], cnt[:], k_t[:], op=Alu.is_le)
                lo2 = wpool.tile([128, W], f32, tag="lo2")
                nc.vector.select(lo2[:], sel[:], mid[:], lo_t[:])
                hi2 = wpool.tile([128, W], f32, tag="hi2")
                nc.vector.select(hi2[:], sel[:], hi_t[:], mid[:])
                nc.vector.tensor_copy(lo_t[:], lo2[:])
                nc.vector.tensor_copy(hi_t[:], hi2[:])

            body = {"v3": sweep_body_v3, "v2": sweep_body_v2}.get(variant, sweep_body)
            if variant == "v3" and approx_tail > 0:
                with tc.For_i(0, sweeps - approx_tail, 1):
                    sweep_body_v3(use_approx=False)
                with tc.For_i(0, approx_tail, 1):
                    sweep_body_v3(use_approx=True)
            else:
                with tc.For_i(0, sweeps, 1):
                    body()

            # ev = (lo+hi)/2
            t1 = wpool.tile([128, W], f32, tag="t0")
            nc.vector.tensor_add(t1[:], lo_t[:], hi_t[:])
            ev_t = wpool.tile([128, W], f32, tag="mid")
            nc.vector.tensor_scalar_mul(ev_t[:], t1[:], 0.5)
            nc.gpsimd.dma_start(EV[:], ev_t[:])

    nc.compile()
    return nc


def _host_inputs(ptl):
    """Build per-core input maps. ptl: (4, 2000) f32."""
    ptl = np.asarray(ptl, np.float32)
    r = np.linspace(RM / RN, RM, RN, dtype=np.float32)
    lv = np.arange(LMAX + 1, dtype=np.float32)
    eff = (lv * (lv + 1.0))[:, None] / (r * r)[None, :]          # (L, RN)
    d = 2.0 * S + ptl[:, None, :] + eff[None]                     # (B, L, RN) f32
    dsc = (d / S).astype(np.float32).reshape(BDIM * (LMAX + 1), RN)  # (16, RN)

    kk = np.minimum(
        (np.arange(PARTS_PER_MAT)[:, None] * TASKS_PER_PART
         + np.arange(TASKS_PER_PART)[None, :]),
        RN - 1,
    ).astype(np.float32)                                          # (64, 32)

    in_maps = []
    for core in range(NCORES):
        Dc = np.empty((128, RN), np.float32)
        LOc = np.empty((128, TASKS_PER_PART), np.float32)
        HIc = np.empty((128, TASKS_PER_PART), np.float32)
        Kc = np.empty((128, TASKS_PER_PART), np.float32)
        for j in range(MATS_PER_CORE):
            mat = MATS_PER_CORE * core + j
            sl = slice(j * PARTS_PER_MAT, (j + 1) * PARTS_PER_MAT)
            Dc[sl] = dsc[mat]
            gl = np.float32(dsc[mat].min() - 2.0)
            gu = np.float32(dsc[mat].max() + 2.0)
            LOc[sl] = gl
            HIc[sl] = gu
            Kc[sl] = kk
        in_maps.append({"d": Dc, "ktgt": Kc, "lo0": LOc, "hi0": HIc})
    return in_maps


def _unshard(results):
    """results: list of 8 out-maps with 'ev' [128, 32] -> (B, L, RN) f32."""
    out = np.empty((BDIM * (LMAX + 1), RN), np.float32)
    for core in range(NCORES):
        ev = results[core]["ev"]                                  # (128, 32)
        for j in range(MATS_PER_CORE):
            mat = MATS_PER_CORE * core + j
            sl = slice(j * PARTS_PER_MAT, (j + 1) * PARTS_PER_MAT)
            out[mat] = ev[sl].reshape(-1)[:RN]
    return (out * S).reshape(BDIM, LMAX + 1, RN)


def kernel(ptl):
    from concourse.bass_utils import run_bass_kernel_spmd

    key = (SWEEPS, RN, VARIANT, APPROX, CHUNK, APPROX_TAIL)
    if key not in _CACHE:
        _CACHE[key] = _build_nc(
            sweeps=SWEEPS, steps=RN, variant=VARIANT, approx=APPROX, chunk=CHUNK,
            approx_tail=APPROX_TAIL,
        )
    nc = _CACHE[key]

    in_maps = _host_inputs(ptl)
    # The axon-tunneled devices occasionally report a transient
    # "exec unit unrecoverable" on the first multi-core launch; retry.
    last_err = None
    for attempt in range(3):
        try:
            res = run_bass_kernel_spmd(nc, in_maps, core_ids=list(range(NCORES)))
            return _unshard(res.results)
        except Exception as e:  # noqa: BLE001
            last_err = e
            import time as _time
            _time.sleep(10.0 * (attempt + 1))
    raise last_err


if __name__ == "__main__":
    x = np.random.RandomState(0).randn(BDIM, RN).astype(np.float32)
    out = kernel(x)
    print(out.shape, out.dtype, out[0, 0, :5])

